# revision 1
# baseline (speedup 1.0000x reference)
"""DeepSeek-style MoE forward on 8 Trainium2 NeuronCores.

Strategy (expert-parallel, per sharding hint):
  Phase 1 (device, data-parallel): each core computes the softmax gate +
    top-2 renormalized weights for its 512-token slice. All gate math
    (matmul, softmax, top-2 select, renorm) runs on device in fp32.
  Host dispatch: tokens are routed to expert cores by the device-computed
    top-k weights (the "all-to-all", emulated with numpy gathers; layout
    transposed to feature-major for the device).
  Phase 2 (device, expert-parallel): core e holds expert e's weights and
    runs the SwiGLU FFN on its gathered tokens, scaling by the routing
    weight on chip. Rare capacity overflow falls back to exact host math.
  Host combine: scatter-add expert outputs + residual.

Self-contained: shapes hardcoded from the problem spec.
"""
import os
import sys

import numpy as np

if "/opt/trn_rl_repo" not in sys.path:
    sys.path.insert(0, "/opt/trn_rl_repo")

import concourse.tile as tile
from concourse import bacc, mybir
from concourse.bass_utils import run_bass_kernel_spmd
from concourse.masks import make_identity

B, S, D, E, H = 2, 2048, 2048, 8, 1024
T = B * S            # 4096 tokens
N_CORES = 8
TPC = T // N_CORES   # 512 tokens/core for the gate phase
CAP = int(os.environ.get("BASS_MOE_CAP", "1152"))  # per-expert capacity
P = 128
KD = D // P          # 16
KH = H // P          # 8
CT = CAP // P        # 9
_cgw = 384 if CAP % 384 == 0 else 512
_c0s = list(range(0, CAP, _cgw))
CGROUPS = [(c0, min(_cgw, CAP - c0)) for c0 in _c0s]
CTGROUPS = [tuple(range(i, min(i + 2, CT))) for i in range(0, CT, 2)]
DG = 512             # down-proj free-dim group
F32 = mybir.dt.float32
F32R = mybir.dt.float32r
F16 = mybir.dt.float16
WDT = {"f32r": F32R, "f16": F16}[os.environ.get("BASS_MOE_DTYPE", "f16")]
WNP = {F32R: np.float32, F16: np.float16}[WDT]
AF = mybir.ActivationFunctionType
OP = mybir.AluOpType
AX = mybir.AxisListType

_gate_nc = None
_moe_nc = None
_wprep_cache = {}
# exec times (ns) of the last kernel() call, when tracing is enabled via
# BASS_KERNEL_TRACE=1 (read by test.py)
LAST_EXEC_NS = {"gate": None, "moe": None}
_TMPDIR = os.environ.get("BASS_KERNEL_TMPDIR")


def _axon_reset():
    """Recover a wedged NeuronCore (NRT_EXEC_UNIT_UNRECOVERABLE) via the
    axon client's reset entry point. Best-effort."""
    try:
        import ctypes

        lib = ctypes.CDLL("/opt/axon/libaxon_pjrt.so")
        lib.axon_reset.restype = ctypes.c_int64
        lib.axon_reset()
    except Exception:
        pass


def _run_spmd(nc, in_maps, trace, tag):
    try:
        return run_bass_kernel_spmd(
            nc, in_maps, core_ids=list(range(N_CORES)), trace=trace,
            tmpdir=(_TMPDIR + "/" + tag) if (trace and _TMPDIR) else None,
        )
    except Exception:
        _axon_reset()
        return run_bass_kernel_spmd(
            nc, in_maps, core_ids=list(range(N_CORES)), trace=trace,
            tmpdir=(_TMPDIR + "/" + tag + "_retry") if (trace and _TMPDIR) else None,
        )


def _build_gate_nc():
    """Gate kernel: per-core 512-token slice -> renormalized top-2 weights.

    Inputs (feature-major, host-transposed layout):
      xst  [P, KD, TPC]  slice of x^T   (xst[p, k, t] = x[t, k*P+p])
      wgt  [P, KD, E]    W_gate^T      (wgt[p, k, e] = W_gate[e, k*P+p])
    Output:
      wout [TPC, E]  w[t, e] = renormalized top-2 weight, 0 if not selected

    scores^T = wgt.T @ x^T is computed with the 8-column gate weight as the
    stationary operand (cheap weight loads), then PE-transposed back to
    token-major for the softmax/top-2 chain.
    """
    nc = bacc.Bacc(None, target_bir_lowering=False, enable_partition_id=False)
    xst = nc.dram_tensor("xst", [P, KD, TPC], F32, kind="ExternalInput")
    wgt = nc.dram_tensor("wgt", [P, KD, E], F32, kind="ExternalInput")
    wout = nc.dram_tensor("wout", [TPC, E], F32, kind="ExternalOutput")

    with tile.TileContext(nc) as tc:
        with (
            tc.tile_pool(name="xp", bufs=1) as xp,
            tc.tile_pool(name="wp", bufs=1) as wp,
            tc.tile_pool(name="psum", bufs=2, space="PSUM") as psum_pool,
            tc.tile_pool(name="v", bufs=2) as vp,
        ):
            ident = wp.tile([P, P], F32)
            make_identity(nc, ident[:])
            wgt_sb = wp.tile([P, KD, E], F32)
            nc.sync.dma_start(wgt_sb[:], wgt[:])
            XCH = 2
            xst_ch = []
            for c in range(KD // XCH):
                t = xp.tile([P, XCH, TPC], F32, tag=f"xst{c}", name=f"xst{c}")
                nc.sync.dma_start(t[:], xst[:, c * XCH:(c + 1) * XCH, :])
                xst_ch.append(t)

            # scores^T [E, TPC], contraction over D in 16 k-tiles
            ps_st = psum_pool.tile([E, TPC], F32, tag="ps_st")
            for k in range(KD):
                nc.tensor.matmul(
                    ps_st[:],
                    lhsT=wgt_sb[:, k, :],
                    rhs=xst_ch[k // XCH][:, k % XCH, :],
                    start=(k == 0),
                    stop=(k == KD - 1),
                )
            st_sb = vp.tile([E, TPC], F32, tag="st")
            nc.vector.tensor_copy(st_sb[:], ps_st[:])

            w_all = vp.tile([P, TPC // P, E], F32, tag="w_all")
            for tt in range(TPC // P):
                ps = psum_pool.tile([P, E], F32, tag="scores")
                nc.tensor.transpose(
                    ps[:], st_sb[:, tt * P:(tt + 1) * P], ident[:E, :E]
                )
                # softmax numerator (exp(s - max)); the denominator cancels
                # in the top-2 renormalization, so it is never computed.
                nmax = vp.tile([P, 1], F32, tag="nmax")
                nc.vector.tensor_reduce(
                    nmax[:], ps[:], op=OP.max, axis=AX.X, negate=True
                )
                es = vp.tile([P, E], F32, tag="es")
                nc.scalar.activation(es[:], ps[:], AF.Exp, bias=nmax[:])
                # top-1
                m1 = vp.tile([P, 1], F32, tag="m1")
                nc.vector.tensor_reduce(m1[:], es[:], op=OP.max, axis=AX.X)
                mask1 = vp.tile([P, E], F32, tag="mask1")
                nc.vector.tensor_scalar(
                    mask1[:], es[:], m1[:], None, op0=OP.is_equal
                )
                # es with top-1 removed
                t1 = vp.tile([P, E], F32, tag="t1")
                nc.vector.tensor_tensor(t1[:], es[:], mask1[:], op=OP.mult)
                ew = vp.tile([P, E], F32, tag="ew")
                nc.vector.tensor_tensor(ew[:], es[:], t1[:], op=OP.subtract)
                # top-2
                m2 = vp.tile([P, 1], F32, tag="m2")
                nc.vector.tensor_reduce(m2[:], ew[:], op=OP.max, axis=AX.X)
                mask2 = vp.tile([P, E], F32, tag="mask2")
                nc.vector.tensor_scalar(
                    mask2[:], ew[:], m2[:], None, op0=OP.is_equal
                )
                masks = vp.tile([P, E], F32, tag="masks")
                nc.vector.tensor_tensor(masks[:], mask1[:], mask2[:], op=OP.add)
                sel = vp.tile([P, E], F32, tag="sel")
                nc.vector.tensor_tensor(sel[:], es[:], masks[:], op=OP.mult)
                # renormalize: w = sel / (m1 + m2)
                den = vp.tile([P, 1], F32, tag="den")
                nc.vector.tensor_tensor(den[:], m1[:], m2[:], op=OP.add)
                rden = vp.tile([P, 1], F32, tag="rden")
                nc.vector.reciprocal(rden[:], den[:])
                nc.vector.tensor_scalar(
                    w_all[:, tt, :], sel[:], rden[:], None, op0=OP.mult
                )
            nc.sync.dma_start(
                wout.rearrange("(tt p) e -> p tt e", p=P), w_all[:]
            )
    nc.compile()
    return nc


def _build_moe_nc():
    """Expert FFN kernel: out[c, :] = wsel[c] * (silu(x_c @ Wg) * (x_c @ Wu)) @ Wd.

    Inputs (host-prepared layouts, all feature/contraction-major):
      xt   [P, KD, CAP]      gathered tokens, feature-major
      wg   [KH, P, KD, P]    w_gate_proj[e] as [m, p, k, h_in]
      wu   [KH, P, KD, P]    same for w_up_proj[e]
      wd   [D//DG, P, KH, DG] w_down_proj[e] as [dg, p, k, d_in]
      wsel [CAP]             per-slot routing weight (0 for padding)
    Output:
      out  [CAP, D]
    """
    nc = bacc.Bacc(None, target_bir_lowering=False, enable_partition_id=False)
    xt = nc.dram_tensor("xt", [P, KD, CAP], WDT, kind="ExternalInput")
    wg = nc.dram_tensor("wg", [KH, P, KD, P], WDT, kind="ExternalInput")
    wu = nc.dram_tensor("wu", [KH, P, KD, P], WDT, kind="ExternalInput")
    wd = nc.dram_tensor("wd", [D // DG, P, KH, DG], WDT, kind="ExternalInput")
    wsel = nc.dram_tensor("wsel", [CAP], F32, kind="ExternalInput")
    out = nc.dram_tensor("out", [CAP, D], F32, kind="ExternalOutput")

    with tile.TileContext(nc) as tc:
        with (
            tc.tile_pool(name="xtp", bufs=1) as xtp,
            tc.tile_pool(name="hhp", bufs=1) as hhp,
            tc.tile_pool(name="wcol", bufs=2) as wcol,
            tc.tile_pool(name="psum", bufs=1, space="PSUM") as psum_pool,
            tc.tile_pool(name="op", bufs=3) as op_pool,
            tc.tile_pool(name="misc", bufs=2) as misc,
        ):
            # m=0 weight columns first in DMA issue order: the very first
            # matmul needs them.
            wg_c0 = wcol.tile([P, KD, P], WDT, tag="wgcol", name="wg_c0")
            nc.sync.dma_start(wg_c0[:], wg[0])
            wu_c0 = wcol.tile([P, KD, P], WDT, tag="wucol", name="wu_c0")
            nc.sync.dma_start(wu_c0[:], wu[0])
            # xt in 4 chunks of 4 k-tiles each: separate tiles so matmuls
            # depend only on the chunk they read (Tile deps are per-tile).
            chunks = [2, 2, 4, 4, 4]
            xt_sb = []
            k0 = 0
            for ci, w in enumerate(chunks):
                t = xtp.tile([P, w, CAP], WDT, tag=f"xt{ci}", name=f"xt{ci}")
                nc.sync.dma_start(t[:], xt[:, k0:k0 + w, :])
                xt_sb += [t[:, j] for j in range(w)]
                k0 += w
            wsel_sb = misc.tile([P, CT], F32, tag="wsel")
            nc.sync.dma_start(wsel_sb[:], wsel.rearrange("(ct p) -> p ct", p=P))

            hh_sb = hhp.tile([P, KH, CAP], WDT)

            # ---- gate/up projections + silu*mul, feature-major [H, CAP] ----
            for m in range(KH):
                if m == 0:
                    wg_col, wu_col = wg_c0, wu_c0
                else:
                    wg_col = wcol.tile([P, KD, P], WDT, tag="wgcol")
                    nc.sync.dma_start(wg_col[:], wg[m])
                    wu_col = wcol.tile([P, KD, P], WDT, tag="wucol")
                    nc.sync.dma_start(wu_col[:], wu[m])
                ps_g = [
                    psum_pool.tile([P, 512], F32, tag=f"ps_g{gi}", name=f"ps_g{gi}")
                    for gi in range(len(CGROUPS))
                ]
                ps_u = [
                    psum_pool.tile([P, 512], F32, tag=f"ps_u{gi}", name=f"ps_u{gi}")
                    for gi in range(len(CGROUPS))
                ]
                for k in range(KD):
                    for gi, (c0, cn) in enumerate(CGROUPS):
                        nc.tensor.matmul(
                            ps_g[gi][:, :cn],
                            lhsT=wg_col[:, k, :],
                            rhs=xt_sb[k][:, c0:c0 + cn],
                            start=(k == 0),
                            stop=(k == KD - 1),
                        )
                    for gi, (c0, cn) in enumerate(CGROUPS):
                        nc.tensor.matmul(
                            ps_u[gi][:, :cn],
                            lhsT=wu_col[:, k, :],
                            rhs=xt_sb[k][:, c0:c0 + cn],
                            start=(k == 0),
                            stop=(k == KD - 1),
                        )
                for gi, (c0, cn) in enumerate(CGROUPS):
                    tmp = misc.tile([P, 512], WDT, tag="silu")
                    nc.scalar.activation(tmp[:, :cn], ps_g[gi][:, :cn], AF.Silu)
                    nc.vector.tensor_tensor(
                        hh_sb[:, m, c0:c0 + cn],
                        tmp[:, :cn],
                        ps_u[gi][:, :cn],
                        op=OP.mult,
                    )

            # ---- down projection, token-major out [CAP, D], fused wsel ----
            for dgi in range(D // DG):
                wd_col = wcol.tile([P, KH, DG], WDT, tag="wdcol")
                nc.sync.dma_start(wd_col[:], wd[dgi])
                for cts in CTGROUPS:
                    ps_o = [
                        psum_pool.tile([P, DG], F32, tag=f"ps_o{j}", name=f"ps_o{j}")
                        for j in range(len(cts))
                    ]
                    for k in range(KH):
                        for j, ct in enumerate(cts):
                            nc.tensor.matmul(
                                ps_o[j][:],
                                lhsT=hh_sb[:, k, ct * P:(ct + 1) * P],
                                rhs=wd_col[:, k, :],
                                start=(k == 0),
                                stop=(k == KH - 1),
                            )
                    for j, ct in enumerate(cts):
                        o_sb = op_pool.tile([P, DG], F32, tag="o")
                        nc.vector.tensor_scalar(
                            o_sb[:], ps_o[j][:], wsel_sb[:, ct:ct + 1], None,
                            op0=OP.mult,
                        )
                        nc.sync.dma_start(
                            out[ct * P:(ct + 1) * P, dgi * DG:(dgi + 1) * DG],
                            o_sb[:],
                        )
    nc.compile()
    return nc


def _feature_major(a2d, dtype=np.float32):
    """[D, N] -> [P, D//P, N] (partition, k-tile, free), contiguous."""
    d, n = a2d.shape
    return np.ascontiguousarray(
        a2d.reshape(d // P, P, n).transpose(1, 0, 2).astype(dtype)
    )


def _host_expert(x_tok, wg_e, wu_e, wd_e):
    """Exact fp32 SwiGLU expert for rare capacity-overflow tokens."""
    g = x_tok @ wg_e
    u = x_tok @ wu_e
    hh = (g / (1.0 + np.exp(-g))) * u
    return hh @ wd_e


def kernel(hidden_states, W_gate, w_gate_proj, w_up_proj, w_down_proj):
    global _gate_nc, _moe_nc
    trace = os.environ.get("BASS_KERNEL_TRACE") == "1"

    hidden_states = np.asarray(hidden_states, dtype=np.float32)
    W_gate = np.asarray(W_gate, dtype=np.float32)
    w_gate_proj = np.asarray(w_gate_proj, dtype=np.float32)
    w_up_proj = np.asarray(w_up_proj, dtype=np.float32)
    w_down_proj = np.asarray(w_down_proj, dtype=np.float32)

    x = np.ascontiguousarray(hidden_states.reshape(T, D))

    if _gate_nc is None:
        _gate_nc = _build_gate_nc()
    if _moe_nc is None:
        _moe_nc = _build_moe_nc()

    # ---- phase 1: gate on device (data-parallel over tokens) ----
    wgt_host = _feature_major(W_gate.T)  # [P, KD, E]
    in_maps1 = []
    for c in range(N_CORES):
        xs = x[c * TPC:(c + 1) * TPC]            # [TPC, D]
        in_maps1.append({
            "xst": _feature_major(xs.T),          # [P, KD, TPC]
            "wgt": wgt_host,
        })
    res1 = _run_spmd(_gate_nc, in_maps1, trace, "gate")
    LAST_EXEC_NS["gate"] = res1.exec_time_ns
    w = np.concatenate([r["wout"] for r in res1.results], axis=0)  # [T, E]

    # ---- host dispatch: route tokens to expert cores ----
    in_maps2 = []
    idx_list = []
    overflow = []  # (expert, token idx array) handled exactly on host
    for e in range(E):
        idx = np.flatnonzero(w[:, e] > 0.0)
        if len(idx) > CAP:
            overflow.append((e, idx[CAP:]))
            idx = idx[:CAP]
        idx_list.append(idx)
        ne = len(idx)
        xt_h = np.zeros((P, KD, CAP), WNP)
        xt_h[:, :, :ne] = _feature_major(
            np.ascontiguousarray(x[idx].T), dtype=WNP
        )
        ws_h = np.zeros((CAP,), np.float32)
        ws_h[:ne] = w[idx, e]
        ck = (
            e, w_gate_proj.ctypes.data, float(w_gate_proj[e, 0, 0]),
            float(w_up_proj[e, 1, 1]), float(w_down_proj[e, 2, 2]),
        )
        if ck not in _wprep_cache:
            _wprep_cache[ck] = (
                np.ascontiguousarray(
                    w_gate_proj[e].reshape(KD, P, KH, P).transpose(2, 1, 0, 3)
                ).astype(WNP),
                np.ascontiguousarray(
                    w_up_proj[e].reshape(KD, P, KH, P).transpose(2, 1, 0, 3)
                ).astype(WNP),
                np.ascontiguousarray(
                    w_down_proj[e].reshape(KH, P, D // DG, DG).transpose(2, 1, 0, 3)
                ).astype(WNP),
            )
        wg_h, wu_h, wd_h = _wprep_cache[ck]
        in_maps2.append({
            "xt": xt_h, "wg": wg_h, "wu": wu_h, "wd": wd_h, "wsel": ws_h,
        })

    # ---- phase 2: expert FFN on device (expert-parallel) ----
    res2 = _run_spmd(_moe_nc, in_maps2, trace, "moe")
    LAST_EXEC_NS["moe"] = res2.exec_time_ns

    # ---- host combine: scatter-add + residual ----
    y = x.copy()
    for e in range(E):
        idx = idx_list[e]
        y[idx] += res2.results[e]["out"][:len(idx)]
    for e, idx in overflow:
        y[idx] += w[idx, e:e + 1] * _host_expert(
            x[idx], w_gate_proj[e], w_up_proj[e], w_down_proj[e]
        ).astype(np.float32)
    return y.reshape(B, S, D)



# revision 5
# speedup vs baseline: 1.1510x; 1.1510x over previous
"""DeepSeek-style MoE forward on 8 Trainium2 NeuronCores.

Strategy (expert-parallel, per sharding hint):
  Phase 1 (device, data-parallel): each core computes the softmax gate +
    top-2 renormalized weights for its 512-token slice, in f16 with a
    sigmoid-based renormalization (w1 = sigmoid(s1 - s2)).
  Host dispatch: tokens are routed to expert cores by the device-computed
    top-k weights (the "all-to-all", emulated with numpy gathers; layout
    transposed to feature-major for the device).
  Phase 2 (device, expert-parallel): core e holds expert e's weights and
    runs the SwiGLU FFN on its gathered tokens; the routing weight is
    folded into hh before the down projection. Rare capacity overflow
    falls back to exact host math.
  Host combine: scatter-add expert outputs + residual.

Perf notes vs the first working version (263 us):
  - f16 everywhere on device (fp32 gate matmuls were 4 cyc/row + cold).
  - CAP 1152 -> 1072 (max expert load is 1063; the 128-alignment that
    forced 1152 is gone: token-dim tiling is (512,512,48) for gate/up
    and (8x128 + 48) for down).
  - PE warm-up matmuls at kernel start flip the HAM clock gate to 8/8
    before the real matmuls arrive.
  - Input DMA ordered so the first matmul's deps land first; wd
    prefetched during the gate/up phase; outputs (f16) issued on the
    Activation-engine DGE queue to avoid head-of-line blocking.
  - Down-proj PSUM triple-buffered (was: 630ns stall per ct-group).

Self-contained: shapes hardcoded from the problem spec.
"""
import os
import sys

import numpy as np

if "/opt/trn_rl_repo" not in sys.path:
    sys.path.insert(0, "/opt/trn_rl_repo")

import concourse.tile as tile
from concourse import bacc, mybir
from concourse.bass_utils import run_bass_kernel_spmd
from concourse.masks import make_identity

B, S, D, E, H = 2, 2048, 2048, 8, 1024
T = B * S            # 4096 tokens
N_CORES = 8
TPC = T // N_CORES   # 512 tokens/core for the gate phase
CAP = int(os.environ.get("BASS_MOE_CAP", "1072"))  # per-expert capacity
P = 128
KD = D // P          # 16
KH = H // P          # 8
# token-dim groups for the gate/up matmuls (N <= 512 per PSUM bank):
# full 512-wide groups double-buffered, plus a narrow tail group that
# accumulates in its own PSUM tile (g in [:, :GU_TAIL_N], u after it)
GU_FULL = [(i * 512, 512) for i in range(CAP // 512)]
GU_TAIL = (CAP - CAP % 512, CAP % 512) if CAP % 512 else None
GU_TAIL_N = GU_TAIL[1] if GU_TAIL else 0
# token tiles for the down projection (stationary operand, <=128 each)
CT_TILES = []
_c0 = 0
while _c0 < CAP:
    CT_TILES.append((_c0, min(P, CAP - _c0)))
    _c0 += P
DG = 512             # down-proj free-dim group
F32 = mybir.dt.float32
F16 = mybir.dt.float16
WNP = np.float16
AF = mybir.ActivationFunctionType
OP = mybir.AluOpType
AX = mybir.AxisListType

_gate_nc = None
_moe_nc = None
_wprep_cache = {}
# exec times (ns) of the last kernel() call, when tracing is enabled via
# BASS_KERNEL_TRACE=1 (read by test.py)
LAST_EXEC_NS = {"gate": None, "moe": None}
_TMPDIR = os.environ.get("BASS_KERNEL_TMPDIR")


def _axon_reset():
    """Recover a wedged NeuronCore (NRT_EXEC_UNIT_UNRECOVERABLE) via the
    axon client's reset entry point. Best-effort."""
    try:
        import ctypes

        lib = ctypes.CDLL("/opt/axon/libaxon_pjrt.so")
        lib.axon_reset.restype = ctypes.c_int64
        lib.axon_reset()
    except Exception:
        pass


def _run_spmd(nc, in_maps, trace, tag):
    try:
        return run_bass_kernel_spmd(
            nc, in_maps, core_ids=list(range(N_CORES)), trace=trace,
            tmpdir=(_TMPDIR + "/" + tag) if (trace and _TMPDIR) else None,
        )
    except Exception:
        _axon_reset()
        return run_bass_kernel_spmd(
            nc, in_maps, core_ids=list(range(N_CORES)), trace=trace,
            tmpdir=(_TMPDIR + "/" + tag + "_retry") if (trace and _TMPDIR) else None,
        )


def _emit_warmup(nc, sbuf_pool, psum_pool, psum_tag, n_mm, width):
    """Dummy matmuls with no DMA deps: keep the PE busy from the moment its
    preamble barrier clears so the HAM clock gate flips to 8/8 before the
    first real matmul (which waits ~3-6us for input DMA)."""
    warm_src = sbuf_pool.tile([P, width], F16, tag="warm_src")
    nc.gpsimd.memset(warm_src[:], 0.0)
    ps = psum_pool.tile([P, width], F32, tag=psum_tag, name="warm_ps")
    for _ in range(n_mm):
        nc.tensor.matmul(
            ps[:], lhsT=warm_src[:, :P], rhs=warm_src[:], start=True, stop=True
        )


def _build_gate_nc():
    """Gate kernel: per-core 512-token slice -> renormalized top-2 weights.

    Inputs (feature-major, host-transposed f16 layout):
      xst  [P, KD, TPC]  slice of x^T   (xst[p, k, t] = x[t, k*P+p])
      wgt  [P, KD, E]    W_gate^T      (wgt[p, k, e] = W_gate[e, k*P+p])
    Output:
      wout [TPC, E]  w[t, e] = renormalized top-2 weight, 0 if not selected

    scores^T = wgt.T @ x^T with the 8-column gate weight stationary, then
    PE-transposed back to token-major. Top-2 renormalization uses
    w1 = sigmoid(s1 - s2), w2 = sigmoid(s2 - s1) (the softmax denominator
    cancels), batched across all 4 token tiles in one 13-op chain.
    """
    nc = bacc.Bacc(None, target_bir_lowering=False, enable_partition_id=False)
    xst = nc.dram_tensor("xst", [P, KD, TPC], F16, kind="ExternalInput")
    wgt = nc.dram_tensor("wgt", [P, KD, E], F16, kind="ExternalInput")
    wout = nc.dram_tensor("wout", [TPC, E], F32, kind="ExternalOutput")
    NTT = TPC // P  # 4 token tiles

    with tile.TileContext(nc) as tc:
        with (
            tc.tile_pool(name="xp", bufs=1) as xp,
            tc.tile_pool(name="wp", bufs=1) as wp,
            tc.tile_pool(name="psum", bufs=2, space="PSUM") as psum_pool,
            tc.tile_pool(name="v", bufs=2) as vp,
        ):
            _emit_warmup(nc, wp, psum_pool, "warm", 10, 512)
            ident = wp.tile([P, P], F32)
            make_identity(nc, ident[:])
            wgt_sb = wp.tile([P, KD, E], F16)
            nc.sync.dma_start(wgt_sb[:], wgt[:])
            XCH = 2
            xst_ch = []
            for c in range(KD // XCH):
                t = xp.tile([P, XCH, TPC], F16, tag=f"xst{c}", name=f"xst{c}")
                nc.sync.dma_start(t[:], xst[:, c * XCH:(c + 1) * XCH, :])
                xst_ch.append(t)

            # scores^T [E, TPC], contraction over D in 16 k-tiles
            ps_st = psum_pool.tile([E, TPC], F32, tag="ps_st")
            for k in range(KD):
                nc.tensor.matmul(
                    ps_st[:],
                    lhsT=wgt_sb[:, k, :],
                    rhs=xst_ch[k // XCH][:, k % XCH, :],
                    start=(k == 0),
                    stop=(k == KD - 1),
                )
            st_sb = vp.tile([E, TPC], F32, tag="st")
            nc.vector.tensor_copy(st_sb[:], ps_st[:])

            # transpose back to token-major: sc [P, NTT, E]
            sc = vp.tile([P, NTT, E], F32, tag="sc")
            for tt in range(NTT):
                ps = psum_pool.tile([P, E], F32, tag="scores")
                nc.tensor.transpose(
                    ps[:], st_sb[:, tt * P:(tt + 1) * P], ident[:E, :E]
                )
                nc.vector.tensor_copy(sc[:, tt, :], ps[:])

            # batched top-2 chain over [P, NTT, E]
            m1 = vp.tile([P, NTT, 1], F32, tag="m1")
            nc.vector.tensor_reduce(m1[:], sc[:], op=OP.max, axis=AX.X)
            mask1 = vp.tile([P, NTT, E], F32, tag="mask1")
            nc.vector.tensor_tensor(
                mask1[:], sc[:], m1[:].broadcast_to([P, NTT, E]), op=OP.is_equal
            )
            knock = vp.tile([P, NTT, E], F32, tag="knock")
            nc.vector.tensor_scalar(
                knock[:], mask1[:], 1e30, None, op0=OP.mult
            )
            sc2 = vp.tile([P, NTT, E], F32, tag="sc2")
            nc.vector.tensor_tensor(sc2[:], sc[:], knock[:], op=OP.subtract)
            m2 = vp.tile([P, NTT, 1], F32, tag="m2")
            nc.vector.tensor_reduce(m2[:], sc2[:], op=OP.max, axis=AX.X)
            mask2 = vp.tile([P, NTT, E], F32, tag="mask2")
            nc.vector.tensor_tensor(
                mask2[:], sc2[:], m2[:].broadcast_to([P, NTT, E]), op=OP.is_equal
            )
            d12 = vp.tile([P, NTT, 1], F32, tag="d12")
            nc.vector.tensor_tensor(d12[:], m1[:], m2[:], op=OP.subtract)
            w1 = vp.tile([P, NTT, 1], F32, tag="w1")
            nc.scalar.activation(w1[:], d12[:], AF.Sigmoid)
            nd12 = vp.tile([P, NTT, 1], F32, tag="nd12")
            nc.vector.tensor_scalar(nd12[:], d12[:], -1.0, None, op0=OP.mult)
            w2 = vp.tile([P, NTT, 1], F32, tag="w2")
            nc.scalar.activation(w2[:], nd12[:], AF.Sigmoid)
            o1 = vp.tile([P, NTT, E], F32, tag="o1")
            nc.vector.tensor_tensor(
                o1[:], mask1[:], w1[:].broadcast_to([P, NTT, E]), op=OP.mult
            )
            w_all = vp.tile([P, NTT, E], F32, tag="w_all")
            nc.vector.tensor_tensor(
                w_all[:], mask2[:], w2[:].broadcast_to([P, NTT, E]), op=OP.mult
            )
            nc.vector.tensor_tensor(w_all[:], w_all[:], o1[:], op=OP.add)
            nc.scalar.dma_start(
                wout.rearrange("(tt p) e -> p tt e", p=P), w_all[:]
            )
    nc.compile()
    return nc


def _build_moe_nc():
    """Expert FFN kernel: out[c, :] = (silu(x_c @ Wg) * (x_c @ Wu) * wsel[c]) @ Wd.

    Inputs (host-prepared f16 layouts, all feature/contraction-major):
      xt    [P, KD, CAP]   gathered tokens, feature-major
      wg    [KH, P, KD, P] w_gate_proj[e] as [m, p, k, h_in]
      wu    [KH, P, KD, P] same for w_up_proj[e]
      wd    [P, KH, D]     w_down_proj[e] feature-major
      wselb [P, CAP]       routing weight per slot, replicated over partitions
    Output:
      out   [CAP, D] f16
    """
    nc = bacc.Bacc(None, target_bir_lowering=False, enable_partition_id=False)
    xt = nc.dram_tensor("xt", [P, KD, CAP], F16, kind="ExternalInput")
    wg = nc.dram_tensor("wg", [KH, P, KD, P], F16, kind="ExternalInput")
    wu = nc.dram_tensor("wu", [KH, P, KD, P], F16, kind="ExternalInput")
    wd = nc.dram_tensor("wd", [P, KH, D], F16, kind="ExternalInput")
    wselb = nc.dram_tensor("wselb", [P, CAP], F16, kind="ExternalInput")
    out = nc.dram_tensor("out", [CAP, D], F16, kind="ExternalOutput")

    with tile.TileContext(nc) as tc:
        with (
            tc.tile_pool(name="xtp", bufs=1) as xtp,
            tc.tile_pool(name="hhp", bufs=1) as hhp,
            tc.tile_pool(name="wcol", bufs=2) as wcol,
            tc.tile_pool(name="psum", bufs=1, space="PSUM") as psum_pool,
            tc.tile_pool(name="pgu", bufs=2, space="PSUM") as pgu_pool,
            tc.tile_pool(name="pd", bufs=3, space="PSUM") as pd_pool,
            tc.tile_pool(name="op", bufs=3) as op_pool,
            tc.tile_pool(name="misc", bufs=2) as misc,
        ):
            _emit_warmup(nc, misc, pd_pool, "pd", 10, DG)

            # DMA issue order = priority order: the first (m=0, k=0) matmul
            # needs the first half of wg[0] and the first xt k-tile.
            wg_c0 = wcol.tile([P, KD, P], F16, tag="wgcol", name="wg_c0")
            nc.sync.dma_start(wg_c0[:, 0:8, :], wg[0, :, 0:8, :])
            chunks = [1, 1, 2, 4, 4, 4]
            xt_sb = []
            xt_dmas = []
            k0 = 0
            for ci, w in enumerate(chunks):
                t = xtp.tile([P, w, CAP], F16, tag=f"xt{ci}", name=f"xt{ci}")
                if ci < 2:
                    nc.sync.dma_start(t[:], xt[:, k0:k0 + w, :])
                else:
                    xt_dmas.append((t, k0, w))
                xt_sb += [t[:, j] for j in range(w)]
                k0 += w
            wu_c0 = wcol.tile([P, KD, P], F16, tag="wucol", name="wu_c0")
            nc.sync.dma_start(wu_c0[:, 0:8, :], wu[0, :, 0:8, :])
            for t, k0_, w in xt_dmas:
                nc.sync.dma_start(t[:], xt[:, k0_:k0_ + w, :])
            nc.sync.dma_start(wg_c0[:, 8:16, :], wg[0, :, 8:16, :])
            nc.sync.dma_start(wu_c0[:, 8:16, :], wu[0, :, 8:16, :])
            wsel_sb = misc.tile([P, CAP], F16, tag="wsel")
            nc.sync.dma_start(wsel_sb[:], wselb[:])

            hh_sb = hhp.tile([P, KH, CAP], F16)
            wd_sb = hhp.tile([P, KH, D], F16, tag="wd_sb", name="wd_sb")

            # ---- gate/up projections + silu*mul*wsel, feature-major ----
            for m in range(KH):
                if m == 0:
                    wg_col, wu_col = wg_c0, wu_c0
                else:
                    wg_col = wcol.tile([P, KD, P], F16, tag="wgcol")
                    nc.sync.dma_start(wg_col[:], wg[m])
                    wu_col = wcol.tile([P, KD, P], F16, tag="wucol")
                    nc.sync.dma_start(wu_col[:], wu[m])
                if m == 1:
                    # prefetch the down-proj weights behind the m=1 columns
                    nc.sync.dma_start(wd_sb[:, 0:4, :], wd[:, 0:4, :])
                elif m == 2:
                    nc.sync.dma_start(wd_sb[:, 4:8, :], wd[:, 4:8, :])
                ps_g = [
                    pgu_pool.tile([P, 512], F32, tag="ps_g", name=f"ps_g{gi}")
                    for gi in range(len(GU_FULL))
                ]
                ps_u = [
                    pgu_pool.tile([P, 512], F32, tag="ps_u", name=f"ps_u{gi}")
                    for gi in range(len(GU_FULL))
                ]
                if GU_TAIL:
                    ps_t = psum_pool.tile([P, 2 * GU_TAIL_N], F32, tag="ps_t")
                def _mm(which, gi, k):
                    # which: 'g'/'u'; gi: group index or 'T' for the tail
                    w_col = wg_col if which == "g" else wu_col
                    if gi == "T":
                        off = 0 if which == "g" else GU_TAIL_N
                        dst = ps_t[:, off:off + GU_TAIL_N]
                        c0, cn = GU_TAIL[0], GU_TAIL_N
                    else:
                        dst = (ps_g if which == "g" else ps_u)[gi][:, :512]
                        c0, cn = GU_FULL[gi]
                    nc.tensor.matmul(
                        dst,
                        lhsT=w_col[:, k, :],
                        rhs=xt_sb[k][:, c0:c0 + cn],
                        start=(k == 0),
                        stop=(k == KD - 1),
                    )

                tail = [("g", "T"), ("u", "T")] if GU_TAIL else []
                for k in range(KD):
                    if k == 0:
                        # tail psum (single buffer) is still being read by
                        # the previous m's silu chain: issue its MMs last
                        order = [("g", 0), ("g", 1), ("u", 0), ("u", 1)] + tail
                    elif k == KD - 1:
                        # finish group 0 (g then u) first so its silu chain
                        # starts while the remaining matmuls stream
                        order = [("g", 0), ("u", 0), ("g", 1), ("u", 1)] + tail
                    else:
                        order = [("g", 0), ("g", 1)] + tail[:1] + \
                                [("u", 0), ("u", 1)] + tail[1:]
                    for which, gi in order:
                        _mm(which, gi, k)
                gu_parts = [
                    (c0, cn, ps_g[gi][:, :cn], ps_u[gi][:, :cn])
                    for gi, (c0, cn) in enumerate(GU_FULL)
                ]
                if GU_TAIL:
                    gu_parts.append((
                        GU_TAIL[0], GU_TAIL_N,
                        ps_t[:, :GU_TAIL_N], ps_t[:, GU_TAIL_N:2 * GU_TAIL_N],
                    ))
                for c0, cn, pg_ap, pu_ap in gu_parts:
                    tmp = misc.tile([P, 512], F16, tag="silu")
                    nc.scalar.activation(tmp[:, :cn], pg_ap, AF.Silu)
                    tmp2 = misc.tile([P, 512], F16, tag="uw")
                    nc.vector.tensor_tensor(
                        tmp2[:, :cn],
                        pu_ap,
                        wsel_sb[:, c0:c0 + cn],
                        op=OP.mult,
                    )
                    nc.vector.tensor_tensor(
                        hh_sb[:, m, c0:c0 + cn],
                        tmp[:, :cn],
                        tmp2[:, :cn],
                        op=OP.mult,
                    )

            # ---- down projection, token-major out [CAP, D] ----
            for dgi in range(D // DG):
                for (t0, tn) in CT_TILES:
                    ps_o = pd_pool.tile([P, DG], F32, tag="pd")
                    for k in range(KH):
                        nc.tensor.matmul(
                            ps_o[:tn, :],
                            lhsT=hh_sb[:, k, t0:t0 + tn],
                            rhs=wd_sb[:, k, dgi * DG:(dgi + 1) * DG],
                            start=(k == 0),
                            stop=(k == KH - 1),
                        )
                    o_sb = op_pool.tile([P, DG], F16, tag="o")
                    nc.vector.tensor_copy(o_sb[:tn, :], ps_o[:tn, :])
                    nc.scalar.dma_start(
                        out[t0:t0 + tn, dgi * DG:(dgi + 1) * DG],
                        o_sb[:tn, :],
                    )
    nc.compile()
    return nc


def _feature_major(a2d, dtype=WNP):
    """[D, N] -> [P, D//P, N] (partition, k-tile, free), contiguous."""
    d, n = a2d.shape
    return np.ascontiguousarray(
        a2d.reshape(d // P, P, n).transpose(1, 0, 2).astype(dtype)
    )


def _host_expert(x_tok, wg_e, wu_e, wd_e):
    """Exact fp32 SwiGLU expert for rare capacity-overflow tokens."""
    g = x_tok @ wg_e
    u = x_tok @ wu_e
    hh = (g / (1.0 + np.exp(-g))) * u
    return hh @ wd_e


def kernel(hidden_states, W_gate, w_gate_proj, w_up_proj, w_down_proj):
    global _gate_nc, _moe_nc
    trace = os.environ.get("BASS_KERNEL_TRACE") == "1"

    hidden_states = np.asarray(hidden_states, dtype=np.float32)
    W_gate = np.asarray(W_gate, dtype=np.float32)
    w_gate_proj = np.asarray(w_gate_proj, dtype=np.float32)
    w_up_proj = np.asarray(w_up_proj, dtype=np.float32)
    w_down_proj = np.asarray(w_down_proj, dtype=np.float32)

    x = np.ascontiguousarray(hidden_states.reshape(T, D))
    xh = x.astype(WNP)

    if _gate_nc is None:
        _gate_nc = _build_gate_nc()
    if _moe_nc is None:
        _moe_nc = _build_moe_nc()

    # ---- phase 1: gate on device (data-parallel over tokens) ----
    wgt_host = _feature_major(W_gate.T.astype(WNP))  # [P, KD, E]
    in_maps1 = []
    for c in range(N_CORES):
        xs = xh[c * TPC:(c + 1) * TPC]            # [TPC, D]
        in_maps1.append({
            "xst": _feature_major(np.ascontiguousarray(xs.T)),  # [P, KD, TPC]
            "wgt": wgt_host,
        })
    res1 = _run_spmd(_gate_nc, in_maps1, trace, "gate")
    LAST_EXEC_NS["gate"] = res1.exec_time_ns
    w = np.concatenate([r["wout"] for r in res1.results], axis=0)  # [T, E]

    # ---- host dispatch: route tokens to expert cores ----
    in_maps2 = []
    idx_list = []
    overflow = []  # (expert, token idx array) handled exactly on host
    for e in range(E):
        idx = np.flatnonzero(w[:, e] > 0.0)
        if len(idx) > CAP:
            overflow.append((e, idx[CAP:]))
            idx = idx[:CAP]
        idx_list.append(idx)
        ne = len(idx)
        xt_h = np.zeros((P, KD, CAP), WNP)
        xt_h[:, :, :ne] = _feature_major(np.ascontiguousarray(xh[idx].T))
        ws_h = np.zeros((CAP,), WNP)
        ws_h[:ne] = w[idx, e]
        wsb_h = np.ascontiguousarray(np.broadcast_to(ws_h, (P, CAP)))
        ck = (
            e, w_gate_proj.ctypes.data, float(w_gate_proj[e, 0, 0]),
            float(w_up_proj[e, 1, 1]), float(w_down_proj[e, 2, 2]),
        )
        if ck not in _wprep_cache:
            _wprep_cache[ck] = (
                np.ascontiguousarray(
                    w_gate_proj[e].reshape(KD, P, KH, P).transpose(2, 1, 0, 3)
                ).astype(WNP),
                np.ascontiguousarray(
                    w_up_proj[e].reshape(KD, P, KH, P).transpose(2, 1, 0, 3)
                ).astype(WNP),
                np.ascontiguousarray(
                    w_down_proj[e].reshape(KH, P, D).transpose(1, 0, 2)
                ).astype(WNP),
            )
        wg_h, wu_h, wd_h = _wprep_cache[ck]
        in_maps2.append({
            "xt": xt_h, "wg": wg_h, "wu": wu_h, "wd": wd_h, "wselb": wsb_h,
        })

    # ---- phase 2: expert FFN on device (expert-parallel) ----
    res2 = _run_spmd(_moe_nc, in_maps2, trace, "moe")
    LAST_EXEC_NS["moe"] = res2.exec_time_ns

    # ---- host combine: scatter-add + residual ----
    y = x.copy()
    for e in range(E):
        idx = idx_list[e]
        y[idx] += res2.results[e]["out"][:len(idx)].astype(np.float32)
    for e, idx in overflow:
        y[idx] += w[idx, e:e + 1] * _host_expert(
            x[idx], w_gate_proj[e], w_up_proj[e], w_down_proj[e]
        ).astype(np.float32)
    return y.reshape(B, S, D)
